# revision 1
# baseline (speedup 1.0000x reference)
"""Causal self-attention Trainium2 kernel — tensor-parallel over heads on 8 NeuronCores.

Problem: B=4, T=2048, C=1024, H=16 heads (head_dim 64), fp32.
Sharding: 2 heads per core. Each core computes qkv projection for its head
columns, full causal attention for its heads, and a partial output
projection (its W_proj rows); partials are summed on host.

Matmuls run in float32r (fast fp32 mode, ~1.6e-4 rel err), softmax in fp32.
"""

import numpy as np

import concourse.bass as bass
import concourse.mybir as mybir
from concourse import bacc
from concourse.tile import TileContext
from concourse.masks import make_identity

# Walrus's redundant-LDWEIGHTS elimination is off by default; enabling it
# measures ~12% faster end-to-end on this kernel with identical results.
import concourse.bass_utils as _bu

if not getattr(_bu, "_ldw_opt_patched", False):
    _orig_run_command = _bu.run_command

    def _run_command_ldw_opt(argv, **kwargs):
        argv = ["--enable-ldw-opt=true" if a == "--enable-ldw-opt=false"
                else a for a in argv]
        return _orig_run_command(argv, **kwargs)

    _bu.run_command = _run_command_ldw_opt
    _bu._ldw_opt_patched = True

F32 = mybir.dt.float32
F32R = mybir.dt.float32r

B, T, C, H = 4, 2048, 1024, 16
HD = 64
NCORES = 8
CT = C // 128          # 8 C-tiles (contraction)
QT = 512               # q tile (free dim of S^T matmuls)
KT = 128               # k tile (partition dim of S^T)
SCALE = 1.0 / np.sqrt(HD)

_CACHED = {}


def build_kernel(b=B, t=T, debug_dump=False):
    """Build the per-core SPMD program. t must be a multiple of 512."""
    assert t % QT == 0
    nq = t // QT           # q-tiles per sequence
    nst = t // 128         # 128-token subtiles per sequence
    bt = b * t

    nc = bacc.Bacc("TRN2", target_bir_lowering=False, debug=False,
                   num_devices=NCORES)

    xT = nc.dram_tensor("xT", [C, bt], F32, kind="ExternalInput")
    wq = nc.dram_tensor("wq", [C, 128], F32, kind="ExternalInput")
    wk = nc.dram_tensor("wk", [C, 128], F32, kind="ExternalInput")
    wv = nc.dram_tensor("wv", [C, 128], F32, kind="ExternalInput")
    wp = nc.dram_tensor("wp", [128, C], F32, kind="ExternalInput")
    bq = nc.dram_tensor("bq", [128, 1], F32, kind="ExternalInput")
    bk = nc.dram_tensor("bk", [128, 1], F32, kind="ExternalInput")
    out = nc.dram_tensor("out", [bt, C], F32, kind="ExternalOutput")
    if debug_dump:
        nst0 = t // 128
        dbg_qT = nc.dram_tensor("dbg_qT", [128, t], F32, kind="ExternalOutput")
        dbg_kT = nc.dram_tensor("dbg_kT", [128, t], F32, kind="ExternalOutput")
        dbg_v = nc.dram_tensor("dbg_v", [128, nst0 * 130], F32,
                               kind="ExternalOutput")
        dbg_yT = nc.dram_tensor("dbg_yT", [128, t], F32, kind="ExternalOutput")
        dbg_es = nc.dram_tensor("dbg_es", [128, 2 * QT], F32,
                                kind="ExternalOutput")
        dbg_ya = nc.dram_tensor("dbg_ya", [65, QT], F32, kind="ExternalOutput")

    with TileContext(nc) as tc:
        with (
            tc.tile_pool(name="const", bufs=1) as constp,
            tc.tile_pool(name="xin", bufs=CT + 1) as xin,
            tc.tile_pool(name="qk", bufs=2) as qkp,
            tc.tile_pool(name="es", bufs=4) as esp,
            tc.tile_pool(name="yt", bufs=2) as ytp,
            tc.tile_pool(name="small", bufs=2) as smallp,
            tc.tile_pool(name="outsb", bufs=3) as outp,
            tc.tile_pool(name="ps_s", bufs=2, space="PSUM") as ps_s,
            tc.tile_pool(name="ps_ya", bufs=2, space="PSUM") as ps_ya,
            tc.tile_pool(name="ps_misc", bufs=2, space="PSUM") as ps_misc,
        ):
            # ---- constants / weights ----
            ident = constp.tile([128, 128], F32, tag="ident")
            make_identity(nc, ident[:])
            wq_sb = constp.tile([128, C], F32R, tag="wq")
            wk_sb = constp.tile([128, C], F32R, tag="wk")
            wv_sb = constp.tile([128, C], F32R, tag="wv")
            wp_sb = constp.tile([128, C], F32R, tag="wp")
            # lhsT layout: [p, ct*128 + m] = W[ct*128 + p, m]
            for ct in range(CT):
                nc.gpsimd.dma_start(
                    out=wk_sb[:, ct * 128:(ct + 1) * 128],
                    in_=wk[ct * 128:(ct + 1) * 128, :],
                )
            for w_dram, w_sb in ((wq, wq_sb), (wv, wv_sb)):
                nc.gpsimd.dma_start(
                    out=w_sb[:].rearrange("p (ct m) -> p ct m", ct=CT),
                    in_=w_dram[:].rearrange("(ct p) m -> p ct m", p=128),
                )
            nc.gpsimd.dma_start(out=wp_sb[:], in_=wp[:])
            bq_sb = constp.tile([128, 1], F32, tag="bq")
            bk_sb = constp.tile([128, 1], F32, tag="bk")
            nc.sync.dma_start(out=bq_sb[:], in_=bq[:])
            nc.sync.dma_start(out=bk_sb[:], in_=bk[:])

            VW = 130
            one_f32 = constp.tile([128, 1], F32, tag="one")
            nc.vector.memset(one_f32[:], 1.0)

            def emit_proj_qt(pbi, yT_tile, pqt):
                # (kept name; called per-qt for the whole previous batch)
                for sj in range(QT // 128):
                    st = pqt * (QT // 128) + sj
                    osb = outp.tile([128, C], F32, tag="osb")
                    for n in range(C // QT):
                        pp = ps_misc.tile([128, QT], F32, tag="m")
                        nc.tensor.matmul(
                            pp[:],
                            yT_tile[:, st * 128:(st + 1) * 128],
                            wp_sb[:, n * QT:(n + 1) * QT],
                            start=True, stop=True)
                        nc.any.tensor_copy(
                            out=osb[:, n * QT:(n + 1) * QT], in_=pp[:])
                    nc.sync.dma_start(
                        out=out[pbi * t + st * 128:
                                pbi * t + (st + 1) * 128, :],
                        in_=osb[:])

            pending = None   # (bi, yT_tile, qt) awaiting projection
            for bi in range(b):
                # v_sb per 128-token subtile: [A(64)|onesA(1)|B(64)|onesB(1)]
                v_sb = qkp.tile([128, nst * VW], F32R, tag="v")
                v_view = v_sb[:].rearrange("p (s w) -> p s w", w=VW)
                nc.vector.tensor_copy(
                    out=v_view[:, :, 64:65],
                    in_=one_f32[:].to_broadcast((128, nst, 1)))
                nc.vector.tensor_copy(
                    out=v_view[:, :, 129:130],
                    in_=one_f32[:].to_broadcast((128, nst, 1)))
                qT_sb = qkp.tile([128, t], F32R, tag="qT")
                kT_sb = qkp.tile([128, t], F32R, tag="kT")
                yT_sb = ytp.tile([128, t], F32R, tag="yT")
                # ================= QKV projection =================
                xts = []
                for ct in range(CT):
                    xt = xin.tile([128, t], F32R, tag="xt")
                    xts.append(xt)
                for colt in range(t // QT):
                    for ct in range(CT):
                        nc.gpsimd.dma_start(
                            out=xts[ct][:, colt * QT:(colt + 1) * QT],
                            in_=xT[ct * 128:(ct + 1) * 128,
                                   bi * t + colt * QT:
                                   bi * t + (colt + 1) * QT],
                        )

                for colt in range(t // QT):
                    csl = slice(colt * QT, (colt + 1) * QT)
                    for w_sb, dst, bias in (
                        (wk_sb, kT_sb, bk_sb), (wq_sb, qT_sb, bq_sb),
                    ):
                        ps = ps_misc.tile([128, QT], F32, tag="m")
                        for ct in range(CT):
                            nc.tensor.matmul(
                                ps[:],
                                w_sb[:, ct * 128:(ct + 1) * 128],
                                xts[ct][:, csl],
                                start=(ct == 0), stop=(ct == CT - 1),
                            )
                        nc.any.tensor_scalar_add(
                            out=dst[:, csl], in0=ps[:], scalar1=bias[:])
                    # V^T for this col tile, then transpose to natural layout
                    ps = ps_misc.tile([128, QT], F32, tag="m")
                    for ct in range(CT):
                        nc.tensor.matmul(
                            ps[:], wv_sb[:, ct * 128:(ct + 1) * 128],
                            xts[ct][:, csl],
                            start=(ct == 0), stop=(ct == CT - 1))
                    vt_col = smallp.tile([128, QT], F32, tag="vtcol")
                    nc.any.tensor_copy(out=vt_col[:], in_=ps[:])
                    for sj in range(QT // 128):
                        st = colt * (QT // 128) + sj
                        vt_ps = ps_misc.tile([128, 128], F32, tag="m")
                        nc.tensor.transpose(
                            vt_ps[:], vt_col[:, sj * 128:(sj + 1) * 128],
                            ident[:])
                        nc.any.tensor_copy(
                            out=v_sb[:, st * VW:st * VW + 64],
                            in_=vt_ps[:, 0:64])
                        nc.any.tensor_copy(
                            out=v_sb[:, st * VW + 65:st * VW + 129],
                            in_=vt_ps[:, 64:128])



                # ================= attention (heads row-paired) =================
                if pending is not None:
                    pb, pyT = pending
                    for pqt in range(nq):
                        emit_proj_qt(pb, pyT, pqt)
                pending = (bi, yT_sb)
                for qt in range(nq):
                    n_k = (qt + 1) * (QT // KT)   # k-tiles of 128
                    q0 = qt * QT
                    yas = [ps_ya.tile([65, QT], F32, tag="ya",
                                      name=f"ya{_h}")
                           for _h in range(2)]
                    for kt in range(n_k):
                        lo = max(0, kt * KT - q0)
                        # S^T for both heads in one array pass: head A on PE
                        # rows 0-63, head B on rows 64-127 (row tiling).
                        sg = ps_s.tile([128, 2 * QT], F32, tag="sg")
                        es = esp.tile([128, 2 * QT], F32R, tag="es")
                        for h in range(2):
                            hsl = slice(h * 64, (h + 1) * 64)
                            nc.tensor.matmul(
                                sg[:, h * QT + lo:(h + 1) * QT],
                                kT_sb[hsl, kt * KT:(kt + 1) * KT],
                                qT_sb[hsl, q0 + lo:q0 + QT],
                                start=True, stop=True,
                            )
                        # exp for both heads in one op; on diagonal tiles
                        # only the causally-reachable cols [lo:] are computed
                        lo_e = max(0, kt * KT - q0)
                        sg_v = sg[:].rearrange("p (h q) -> p h q", h=2)
                        es_v = es[:].rearrange("p (h q) -> p h q", h=2)
                        nc.scalar.activation(
                            es_v[:, :, lo_e:], sg_v[:, :, lo_e:],
                            mybir.ActivationFunctionType.Exp, scale=SCALE)
                        if kt * KT >= q0:
                            # causal band select, both heads in one op
                            ev = es[:].rearrange("p (h q) -> p h q", h=2)
                            nc.gpsimd.affine_select(
                                out=ev[:, :, lo:lo + KT],
                                in_=ev[:, :, lo:lo + KT],
                                compare_op=mybir.AluOpType.is_ge,
                                fill=0.0,
                                base=0,
                                channel_multiplier=-1,
                                pattern=[[0, 2], [1, KT]],
                            )
                        for h in range(2):
                            nc.tensor.matmul(
                                yas[h][:, lo:QT],
                                v_sb[:, kt * VW + 65 * h:
                                     kt * VW + 65 * h + 65],
                                es[:, h * QT + lo:(h + 1) * QT],
                                start=(kt == 0), stop=(kt == n_k - 1),
                            )
                    for h in range(2):
                        ya = yas[h]
                        # evacuate fast to release the PSUM accumulator
                        ya_sb = smallp.tile([64, QT], F32, tag="yasb")
                        nc.vector.tensor_copy(out=ya_sb[:], in_=ya[0:64, :])
                        srow = smallp.tile([1, QT], F32, tag="srow")
                        nc.scalar.activation(
                            srow[0:1, :], ya[64:65, :],
                            mybir.ActivationFunctionType.Copy)
                        rr = smallp.tile([1, QT], F32, tag="rr")
                        nc.vector.reciprocal_approx_fast(
                            out=rr[0:1, :], in_=srow[0:1, :])
                        # broadcast recip across 64 partitions (SBUF->SBUF)
                        bc = smallp.tile([64, QT], F32, tag="bc")
                        nc.gpsimd.partition_broadcast(
                            bc[:], rr[0:1, :], channels=64)
                        if h == 0:
                            nc.vector.tensor_mul(
                                out=yT_sb[0:64, q0:q0 + QT],
                                in0=ya_sb[:], in1=bc[:])
                        else:
                            ytb = smallp.tile([64, QT], F32R, tag="ytb")
                            nc.vector.tensor_mul(
                                out=ytb[:], in0=ya_sb[:], in1=bc[:])
                            nc.sync.dma_start(
                                out=yT_sb[64:128, q0:q0 + QT], in_=ytb[:])

            pb, pyT = pending
            for pqt in range(nq):
                emit_proj_qt(pb, pyT, pqt)

    nc.compile()
    return nc


def _prep_inputs(x, W_attn, b_attn, W_proj, b_proj, b, t):
    xT_full = np.ascontiguousarray(
        x.reshape(b * t, C).T).astype(np.float32)
    in_maps = []
    for c in range(NCORES):
        sl = slice(c * 128, (c + 1) * 128)
        in_maps.append({
            "xT": xT_full,
            "wq": np.ascontiguousarray(W_attn[:, sl]),
            "wk": np.ascontiguousarray(W_attn[:, 1024:2048][:, sl]),
            "wv": np.ascontiguousarray(W_attn[:, 2048:3072][:, sl]),
            "wp": np.ascontiguousarray(W_proj[sl, :]),
            "bq": np.ascontiguousarray(b_attn[sl].reshape(128, 1)),
            "bk": np.ascontiguousarray(b_attn[1024:2048][sl].reshape(128, 1)),
        })
    return in_maps


def kernel(x, W_attn, b_attn, W_proj, b_proj, _trace=False):
    from concourse.bass_utils import run_bass_kernel_spmd

    x = np.asarray(x, dtype=np.float32)
    W_attn = np.asarray(W_attn, dtype=np.float32)
    b_attn = np.asarray(b_attn, dtype=np.float32)
    W_proj = np.asarray(W_proj, dtype=np.float32)
    b_proj = np.asarray(b_proj, dtype=np.float32)
    b, t, c = x.shape

    key = (b, t)
    if key not in _CACHED:
        _CACHED[key] = build_kernel(b, t)
    nc = _CACHED[key]

    in_maps = _prep_inputs(x, W_attn, b_attn, W_proj, b_proj, b, t)
    res = run_bass_kernel_spmd(
        nc, in_maps, core_ids=list(range(NCORES)), trace=_trace)

    acc = res.results[0]["out"].astype(np.float32).copy()
    for r in res.results[1:]:
        acc += r["out"]
    acc += b_attn[2048:3072] @ W_proj + b_proj
    out = acc.reshape(b, t, c)
    if _trace:
        kernel.last_result = res
    return out

